# revision 7
# baseline (speedup 1.0000x reference)
"""Inverse 2x2 Haar wavelet transform on 8 Trainium2 NeuronCores.

Full inputs:  ll (16, 64, 128, 128) f32, hf (16, 192, 128, 128) f32
Full output:  (16, 64, 256, 256) f32

Sharding: pure data-parallel over batch; core i gets batches [2i, 2i+2).

Precision: the op is a fixed +/-1 butterfly of 4 subbands; end-to-end
fp16 (inputs quantized on host, fp16 compute, fp16 output upcast on
host) gives rel-err ~4e-4, far under the 2e-2 gate, and halves device
HBM traffic: 16 MiB in + 16 MiB out per core vs 64 MiB total in f32.

Per-core kernel: raw Bass 4-engine pipeline, double-buffered.
  SP     issues the 4 input DMAs per channel-group (ll + 3 hf subbands),
  DVE    butterfly stage 1 (t1=ll-lh, t2=hl-hh, s1=ll+lh, s2=hl+hh; all
         contiguous fp16 -> 2x perf mode) and the interleaved writes of
         a and c (stride-2 free-dim views -> 1x mode),
  GPSIMD the interleaved writes of b and d (strided writes drop DVE to
         1x mode anyway, so giving half of them to the otherwise idle
         pool engine halves the 1x work on the DVE critical path),
  ACT    stores OUT with one fully-contiguous DMA per group.

Raw semaphores (not Tile) because TRN2 instructions hold at most one
sync-wait; standalone wait_ge instructions sidestep that cap.

Tile layout: partition p of a group's tile holds G consecutive rows of the
flat (G*H, W) row space (channel boundaries align with partitions), so
input DMAs and the output DMA are fully contiguous per partition.
"""

import os
import sys

import numpy as np

# Make concourse importable in a bare environment without shadowing the
# ambient PYTHONPATH (the axon jax plugin lives in /root/.axon_site).
for _p in (
    "/root/.axon_site",
    "/root/.axon_site/_ro/trn_rl_repo",
    "/root/.axon_site/_ro/pypackages",
    "/opt/trn_rl_repo",
):
    if _p not in sys.path and os.path.isdir(_p):
        sys.path.append(_p)

from concourse import bass, mybir
from concourse.bass_utils import run_bass_kernel_spmd

N_CORES = 8
B, C, H, W = 16, 64, 128, 128
B_LOC = B // N_CORES


def build_haar_nc(B_loc=B_LOC, C=C, H=H, W=W, G=16, NBUF=2, dt=None, split_out=False):
    P = 128
    assert H == P and C % G == 0
    if dt is None:
        dt = mybir.dt.float16
    sub = mybir.AluOpType.subtract
    add = mybir.AluOpType.add

    nc = bass.Bass()
    ll_ext = nc.dram_tensor("ll", [B_loc, C, H, W], dt, kind="ExternalInput")
    hf_ext = nc.dram_tensor("hf", [B_loc, 3 * C, H, W], dt, kind="ExternalInput")
    out_ext = nc.dram_tensor("out", [B_loc, C, 2 * H, 2 * W], dt, kind="ExternalOutput")

    groups = [(b, c0) for b in range(B_loc) for c0 in range(0, C, G)]
    OUT_DMAS = 2 if split_out else 1
    # (C, 3, H, W) DRAM views of each batch's stacked subbands
    hf4 = [hf_ext[b].rearrange("(c s) h w -> c s h w", s=3) for b in range(B_loc)]

    from contextlib import ExitStack

    with ExitStack() as ctx:
        block = ctx.enter_context(nc.Block())
        # Per-buffer-slot DMA sems: completions of different DMAs are
        # unordered, so a single cumulative counter could reach a group's
        # threshold while one of that group's DMAs is still in flight.
        # Same-slot groups ARE ordered (slot reuse waits on s_dve/s_out),
        # so per-slot cumulative thresholds are exact.
        s_in = [ctx.enter_context(nc.semaphore(f"s_in{i}")) for i in range(NBUF)]
        s_dve = ctx.enter_context(nc.semaphore("s_dve"))
        s_gp = ctx.enter_context(nc.semaphore("s_gp"))
        s_out = [ctx.enter_context(nc.semaphore(f"s_out{i}")) for i in range(NBUF)]
        LLb, HFb, OUTb, T1b, T2b, S1b, S2b = [], [], [], [], [], [], []
        for i in range(NBUF):
            LLb.append(ctx.enter_context(nc.sbuf_tensor(f"LL{i}", [P, G, W], dt)))
            HFb.append(ctx.enter_context(nc.sbuf_tensor(f"HF{i}", [P, 3, G, W], dt)))
            OUTb.append(
                ctx.enter_context(nc.sbuf_tensor(f"OUT{i}", [P, G, 2, W, 2], dt))
            )
            T1b.append(ctx.enter_context(nc.sbuf_tensor(f"T1_{i}", [P, G, W], dt)))
            T2b.append(ctx.enter_context(nc.sbuf_tensor(f"T2_{i}", [P, G, W], dt)))
            S1b.append(ctx.enter_context(nc.sbuf_tensor(f"S1_{i}", [P, G, W], dt)))
            S2b.append(ctx.enter_context(nc.sbuf_tensor(f"S2_{i}", [P, G, W], dt)))

        @block.sync
        def _(sync: bass.BassEngine):
            for g, (b, c0) in enumerate(groups):
                if g >= NBUF:
                    # DVE stage 1 of group g-NBUF done -> LL/HF slot free
                    sync.wait_ge(s_dve, 6 * (g - NBUF) + 4)
                i = g % NBUF
                sync.dma_start(out=LLb[i][:], in_=ll_ext[b, c0 : c0 + G]).then_inc(
                    s_in[i], 16
                )
                for s in range(3):
                    sync.dma_start(
                        out=HFb[i][:, s], in_=hf4[b][c0 : c0 + G, s]
                    ).then_inc(s_in[i], 16)

        # DVE op order: T1, T2, a, S1, S2, c — the (a, b) half of the
        # butterfly completes early so the even-row output DMA can start
        # while the odd half is still being computed.
        @block.vector
        def _(vector: bass.BassEngine):
            for g, (b, c0) in enumerate(groups):
                i = g % NBUF
                vector.wait_ge(s_in[i], 64 * (g // NBUF + 1))
                if g >= NBUF:
                    # ACT flushed OUT slot; GPSIMD done reading slot's T/S
                    vector.wait_ge(s_out[i], 16 * OUT_DMAS * (g // NBUF))
                    vector.wait_ge(s_gp, 2 * (g - NBUF + 1))
                LL, HF, OUT = LLb[i], HFb[i], OUTb[i]
                T1, T2, S1, S2 = T1b[i], T2b[i], S1b[i], S2b[i]
                LH, HL, HH = HF[:, 0], HF[:, 1], HF[:, 2]
                vector.tensor_tensor(T1[:], LL[:], LH, sub).then_inc(s_dve, 1)
                vector.tensor_tensor(T2[:], HL, HH, sub).then_inc(s_dve, 1)
                # DVE has no internal RAW interlock: wait for our own
                # completions before consuming T/S tiles.
                vector.wait_ge(s_dve, 6 * g + 2)
                vector.tensor_tensor(OUT[:, :, 0, :, 0], T1[:], T2[:], sub).then_inc(
                    s_dve, 1
                )
                vector.tensor_tensor(S1[:], LL[:], LH, add).then_inc(s_dve, 1)
                vector.tensor_tensor(S2[:], HL, HH, add).then_inc(s_dve, 1)
                vector.wait_ge(s_dve, 6 * g + 5)
                vector.tensor_tensor(OUT[:, :, 1, :, 0], S1[:], S2[:], sub).then_inc(
                    s_dve, 1
                )

        @block.gpsimd
        def _(gpsimd: bass.BassEngine):
            for g, (b, c0) in enumerate(groups):
                i = g % NBUF
                if g >= NBUF:
                    gpsimd.wait_ge(s_out[i], 16 * OUT_DMAS * (g // NBUF))
                OUT = OUTb[i]
                T1, T2, S1, S2 = T1b[i], T2b[i], S1b[i], S2b[i]
                gpsimd.wait_ge(s_dve, 6 * g + 2)
                gpsimd.tensor_tensor(OUT[:, :, 0, :, 1], T1[:], T2[:], add).then_inc(
                    s_gp, 1
                )
                gpsimd.wait_ge(s_dve, 6 * g + 5)
                gpsimd.tensor_tensor(OUT[:, :, 1, :, 1], S1[:], S2[:], add).then_inc(
                    s_gp, 1
                )

        @block.scalar
        def _(scalar: bass.BassEngine):
            for g, (b, c0) in enumerate(groups):
                i = g % NBUF
                # out_ext[b, c0:c0+G] as (c, h, i, w2): i=0 rows hold the
                # interleaved (a,b) halves, i=1 rows hold (c,d).
                dst = out_ext[b, c0 : c0 + G].rearrange(
                    "c (h i) w -> c h i w", i=2
                )
                if split_out:
                    scalar.wait_ge(s_dve, 6 * g + 3)
                    scalar.wait_ge(s_gp, 2 * g + 1)
                    scalar.dma_start(
                        out=dst[:, :, 0], in_=OUTb[i][:, :, 0]
                    ).then_inc(s_out[i], 16)
                    scalar.wait_ge(s_dve, 6 * (g + 1))
                    scalar.wait_ge(s_gp, 2 * (g + 1))
                    scalar.dma_start(
                        out=dst[:, :, 1], in_=OUTb[i][:, :, 1]
                    ).then_inc(s_out[i], 16)
                else:
                    scalar.wait_ge(s_dve, 6 * (g + 1))
                    scalar.wait_ge(s_gp, 2 * (g + 1))
                    scalar.dma_start(
                        out=out_ext[b, c0 : c0 + G], in_=OUTb[i][:]
                    ).then_inc(s_out[i], 16)

    return nc


_NC_CACHE = {}


def _get_nc():
    if "nc" not in _NC_CACHE:
        _NC_CACHE["nc"] = build_haar_nc()
    return _NC_CACHE["nc"]


def _in_maps(ll16: np.ndarray, hf16: np.ndarray) -> list[dict]:
    return [
        {
            "ll": ll16[i * B_LOC : (i + 1) * B_LOC],
            "hf": hf16[i * B_LOC : (i + 1) * B_LOC],
        }
        for i in range(N_CORES)
    ]


def _to_f16(ll: np.ndarray, hf: np.ndarray):
    return (
        np.ascontiguousarray(ll).astype(np.float16),
        np.ascontiguousarray(hf).astype(np.float16),
    )


def kernel(ll: np.ndarray, hf: np.ndarray) -> np.ndarray:
    ll16, hf16 = _to_f16(ll, hf)
    nc = _get_nc()
    res = run_bass_kernel_spmd(nc, _in_maps(ll16, hf16), list(range(N_CORES))).results
    out16 = np.concatenate([res[i]["out"] for i in range(N_CORES)], axis=0)
    return out16.astype(np.float32)


# revision 13
# speedup vs baseline: 2.4301x; 2.4301x over previous
"""Inverse 2x2 Haar wavelet transform on 8 Trainium2 NeuronCores.

Full inputs:  ll (16, 64, 128, 128) f32, hf (16, 192, 128, 128) f32
Full output:  (16, 64, 256, 256) f32

Sharding: pure data-parallel over batch; core i gets batches [2i, 2i+2).

Precision: the op is a fixed +/-1 butterfly of 4 subbands; end-to-end
fp16 (inputs quantized on host, fp16 compute, fp16 output upcast on
host) gives rel-err ~4e-4, far under the 2e-2 gate, and halves device
HBM traffic: 16 MiB in + 16 MiB out per core vs 64 MiB total in f32.

Per-core kernel: raw Bass 4-engine pipeline, double-buffered.
  SP     issues the 4 input DMAs per channel-group (ll + 3 hf subbands),
  DVE    butterfly stage 1 (t1=ll-lh, t2=hl-hh, s1=ll+lh, s2=hl+hh; all
         contiguous fp16 -> 2x perf mode) and the interleaved writes of
         a and c (stride-2 free-dim views -> 1x mode),
  GPSIMD the interleaved writes of b and d (strided writes drop DVE to
         1x mode anyway, so giving half of them to the otherwise idle
         pool engine halves the 1x work on the DVE critical path),
  ACT    stores OUT with one fully-contiguous DMA per group.

Raw semaphores (not Tile) because TRN2 instructions hold at most one
sync-wait; standalone wait_ge instructions sidestep that cap.

Tile layout: partition p of a group's tile holds G consecutive rows of the
flat (G*H, W) row space (channel boundaries align with partitions), so
input DMAs and the output DMA are fully contiguous per partition.
"""

import os
import sys

import numpy as np

# Make concourse importable in a bare environment without shadowing the
# ambient PYTHONPATH (the axon jax plugin lives in /root/.axon_site).
for _p in (
    "/root/.axon_site",
    "/root/.axon_site/_ro/trn_rl_repo",
    "/root/.axon_site/_ro/pypackages",
    "/opt/trn_rl_repo",
):
    if _p not in sys.path and os.path.isdir(_p):
        sys.path.append(_p)

from concourse import bass, mybir
from concourse.bass_utils import run_bass_kernel_spmd

N_CORES = 8
B, C, H, W = 16, 64, 128, 128
B_LOC = B // N_CORES


def build_haar_nc(
    B_loc=B_LOC, C=C, H=H, W=W, G=16, NBUF=2, dt=None, split_out=False, gp_d=None
):
    P = 128
    assert H == P and C % G == 0
    if dt is None:
        dt = mybir.dt.float16
    sub = mybir.AluOpType.subtract
    add = mybir.AluOpType.add

    nc = bass.Bass()
    ll_ext = nc.dram_tensor("ll", [B_loc, C, H, W], dt, kind="ExternalInput")
    hf_ext = nc.dram_tensor("hf", [B_loc, 3 * C, H, W], dt, kind="ExternalInput")
    out_ext = nc.dram_tensor("out", [B_loc, C, 2 * H, 2 * W], dt, kind="ExternalOutput")

    groups = [(b, c0) for b in range(B_loc) for c0 in range(0, C, G)]
    OUT_DMAS = 2 if split_out else 1
    # (C, 3, H, W) DRAM views of each batch's stacked subbands
    hf4 = [hf_ext[b].rearrange("(c s) h w -> c s h w", s=3) for b in range(B_loc)]

    from contextlib import ExitStack

    with ExitStack() as ctx:
        block = ctx.enter_context(nc.Block())
        # Per-buffer-slot DMA sems: completions of different DMAs are
        # unordered, so a single cumulative counter could reach a group's
        # threshold while one of that group's DMAs is still in flight.
        # Same-slot groups ARE ordered (slot reuse waits on s_dve/s_out),
        # so per-slot cumulative thresholds are exact.
        s_in = [ctx.enter_context(nc.semaphore(f"s_in{i}")) for i in range(NBUF)]
        s_dve = ctx.enter_context(nc.semaphore("s_dve"))
        s_gp = ctx.enter_context(nc.semaphore("s_gp"))
        s_out = [ctx.enter_context(nc.semaphore(f"s_out{i}")) for i in range(NBUF)]
        LLb, HFb, OUTb, T1b, T2b, S1b, S2b = [], [], [], [], [], [], []
        for i in range(NBUF):
            LLb.append(ctx.enter_context(nc.sbuf_tensor(f"LL{i}", [P, G, W], dt)))
            HFb.append(ctx.enter_context(nc.sbuf_tensor(f"HF{i}", [P, 3, G, W], dt)))
            OUTb.append(
                ctx.enter_context(nc.sbuf_tensor(f"OUT{i}", [P, G, 2, W, 2], dt))
            )
            T1b.append(ctx.enter_context(nc.sbuf_tensor(f"T1_{i}", [P, G, W], dt)))
            T2b.append(ctx.enter_context(nc.sbuf_tensor(f"T2_{i}", [P, G, W], dt)))
            S1b.append(ctx.enter_context(nc.sbuf_tensor(f"S1_{i}", [P, G, W], dt)))
            S2b.append(ctx.enter_context(nc.sbuf_tensor(f"S2_{i}", [P, G, W], dt)))

        @block.sync
        def _(sync: bass.BassEngine):
            for g, (b, c0) in enumerate(groups):
                if g >= NBUF:
                    # DVE stage 1 of group g-NBUF done (S2 is the 5th inc)
                    # -> LL/HF slot free
                    sync.wait_ge(s_dve, 7 * (g - NBUF) + 5)
                i = g % NBUF
                sync.dma_start(out=LLb[i][:], in_=ll_ext[b, c0 : c0 + G]).then_inc(
                    s_in[i], 16
                )
                for s in range(3):
                    sync.dma_start(
                        out=HFb[i][:, s], in_=hf4[b][c0 : c0 + G, s]
                    ).then_inc(s_in[i], 16)

        # Work split: stride-2 interleaved writes run at 1x DVE mode, so the
        # otherwise-idle GPSIMD takes b and the first gp_d channel-rows of
        # d; DVE keeps stage 1 (contiguous fp16 -> 2x mode) plus a, c, and
        # the rest of d.  Both engines stay under the 93.2 us DMA roofline
        # with margin for model error in either engine's throughput.
        G2 = gp_d if gp_d is not None else G // 2  # GPSIMD takes d[:, :G2]

        @block.vector
        def _(vector: bass.BassEngine):
            for g, (b, c0) in enumerate(groups):
                i = g % NBUF
                vector.wait_ge(s_in[i], 64 * (g // NBUF + 1))
                if g >= NBUF:
                    # ACT flushed OUT slot; GPSIMD done reading slot's T/S
                    vector.wait_ge(s_out[i], 16 * OUT_DMAS * (g // NBUF))
                    vector.wait_ge(s_gp, 2 * (g - NBUF + 1))
                LL, HF, OUT = LLb[i], HFb[i], OUTb[i]
                T1, T2, S1, S2 = T1b[i], T2b[i], S1b[i], S2b[i]
                LH, HL, HH = HF[:, 0], HF[:, 1], HF[:, 2]
                vector.tensor_tensor(T1[:], LL[:], LH, sub).then_inc(s_dve, 1)
                vector.tensor_tensor(T2[:], HL, HH, sub).then_inc(s_dve, 1)
                # DVE has no internal RAW interlock: wait for our own
                # completions before consuming T/S tiles.
                vector.wait_ge(s_dve, 7 * g + 2)
                vector.tensor_tensor(OUT[:, :, 0, :, 0], T1[:], T2[:], sub).then_inc(
                    s_dve, 1
                )
                vector.tensor_tensor(S1[:], LL[:], LH, add).then_inc(s_dve, 1)
                vector.tensor_tensor(S2[:], HL, HH, add).then_inc(s_dve, 1)
                vector.wait_ge(s_dve, 7 * g + 5)
                vector.tensor_tensor(OUT[:, :, 1, :, 0], S1[:], S2[:], sub).then_inc(
                    s_dve, 1
                )
                vector.tensor_tensor(
                    OUT[:, G2:, 1, :, 1], S1[:, G2:], S2[:, G2:], add
                ).then_inc(s_dve, 1)

        @block.gpsimd
        def _(gpsimd: bass.BassEngine):
            for g, (b, c0) in enumerate(groups):
                i = g % NBUF
                if g >= NBUF:
                    gpsimd.wait_ge(s_out[i], 16 * OUT_DMAS * (g // NBUF))
                OUT = OUTb[i]
                T1, T2, S1, S2 = T1b[i], T2b[i], S1b[i], S2b[i]
                gpsimd.wait_ge(s_dve, 7 * g + 2)
                gpsimd.tensor_tensor(OUT[:, :, 0, :, 1], T1[:], T2[:], add).then_inc(
                    s_gp, 1
                )
                gpsimd.wait_ge(s_dve, 7 * g + 5)
                gpsimd.tensor_tensor(
                    OUT[:, :G2, 1, :, 1], S1[:, :G2], S2[:, :G2], add
                ).then_inc(s_gp, 1)

        @block.scalar
        def _(scalar: bass.BassEngine):
            for g, (b, c0) in enumerate(groups):
                i = g % NBUF
                if split_out:
                    # out_ext[b, c0:c0+G] as (c, h, i, w2): i=0 rows hold
                    # the interleaved (a,b) halves, i=1 rows hold (c,d).
                    dst = out_ext[b, c0 : c0 + G].rearrange(
                        "c (h i) w -> c h i w", i=2
                    )
                    scalar.wait_ge(s_dve, 7 * g + 3)
                    scalar.wait_ge(s_gp, 2 * g + 1)
                    scalar.dma_start(
                        out=dst[:, :, 0], in_=OUTb[i][:, :, 0]
                    ).then_inc(s_out[i], 16)
                    scalar.wait_ge(s_dve, 7 * (g + 1))
                    scalar.wait_ge(s_gp, 2 * (g + 1))
                    scalar.dma_start(
                        out=dst[:, :, 1], in_=OUTb[i][:, :, 1]
                    ).then_inc(s_out[i], 16)
                else:
                    scalar.wait_ge(s_dve, 7 * (g + 1))
                    scalar.wait_ge(s_gp, 2 * (g + 1))
                    scalar.dma_start(
                        out=out_ext[b, c0 : c0 + G], in_=OUTb[i][:]
                    ).then_inc(s_out[i], 16)

    return nc


_NC_CACHE = {}


def _get_nc():
    if "nc" not in _NC_CACHE:
        # G=16, NBUF=4: DMA_ENGINES at 100% occupancy in TimelineSim --
        # 93.2 us of transfer (32 MiB @ 360 GB/s) + ~3 us launch/drain.
        _NC_CACHE["nc"] = build_haar_nc(G=16, NBUF=4, split_out=False)
    return _NC_CACHE["nc"]


def _in_maps(ll16: np.ndarray, hf16: np.ndarray) -> list[dict]:
    return [
        {
            "ll": ll16[i * B_LOC : (i + 1) * B_LOC],
            "hf": hf16[i * B_LOC : (i + 1) * B_LOC],
        }
        for i in range(N_CORES)
    ]


def _to_f16(ll: np.ndarray, hf: np.ndarray):
    return (
        np.ascontiguousarray(ll).astype(np.float16),
        np.ascontiguousarray(hf).astype(np.float16),
    )


def kernel(ll: np.ndarray, hf: np.ndarray) -> np.ndarray:
    ll16, hf16 = _to_f16(ll, hf)
    nc = _get_nc()
    res = run_bass_kernel_spmd(nc, _in_maps(ll16, hf16), list(range(N_CORES))).results
    out16 = np.concatenate([res[i]["out"] for i in range(N_CORES)], axis=0)
    return out16.astype(np.float32)
